# revision 61
# baseline (speedup 1.0000x reference)
"""PointerNet additive-attention scores kernel for Trainium2 (8 NeuronCores).

Math (reference):
    kt[k,n,h] = key[k,n,:] @ w1_w[h,:]
    vt[v,n,h] = value[v,n,:] @ w2_w[h,:] + (w1_b[h] + w2_b[h])
    xi[k,v,n] = sum_h v_w[h] * tanh(kt[k,n,h] + vt[v,n,h]) + v_b
    S[k,n]    = sum_v exp(xi[k,v,n]) * mask[v,n];  S==0 -> 1
    out[k,n,v] = xi[k,v,n] - log(S[k,n])

Key trick: tanh(u) on the data range |u| <= 6.9 is approximated by a sum of
M=5 sines, tanh(u) ~ sum_m b_m sin(w_m u), fitted (weighted by the empirical
u-density) to ~9e-4. Since sin(w(k+v)) = sin(wk)cos(wv) + cos(wk)sin(wv),
the (Lk x Lv x H) inner broadcast+tanh collapses to 2M rank-H matmuls on the
PE array. Per-element work drops from Lk*Lv*N*H tanh evals (the ACT-engine
roofline of the direct kernel, ~25 us) to (Lk+Lv)*N*H*2M trig evals.

Per-core dataflow (data-parallel over batch N, NLOC=2 items/core):
  - PE: kt/vt marginals ([128h, (n k)] per h-chunk) from bf16 inputs.
  - DVE range reduction in turns (HW has no mod op): t = x*w/2pi (bf16),
    y = bf16(t + 192) = 192 + round(t) (bf16 magic: ulp(192)=1), and the
    f32 difference d = t - y = frac(t) - 192; the cos branch pre-shifts
    by +0.25 turns. m=0 needs no reduction (|w0 x| + pi/2 < pi).
  - ACT (Sin table): one [128, 2048] instruction per m computes
    sin(2pi*c*d + 2pi*192*c) = sin/cos(w_m x) for both sides, both
    h-chunks, both batch items (scale/bias absorb the -192 offset; c
    slightly < 1 keeps args inside the table's [-pi, pi] domain).
  - DVE: fold v_w[h] * b_m into the v-side factors (tensor_scalar with
    per-partition v_w column and immediate b_m).
  - PE: xi[:, n, :] += s~_k^T (vwb c~_v) + c~_k^T (vwb s~_v), accumulated
    in one PSUM bank over all (m, hc), seeded with v_b via a c=1 matmul.
  - Epilogue: masked sum via log-mask add (0 / -30000) + Exp with
    accum_out (row sums on the fly), S==0 guard, ACT Ln, per-partition
    subtract, contiguous DMA out. Exp/Ln share one table
    (natural_log_exp_and_others), so only two ACT table loads total, both
    hidden under DMA/PE work.
"""

import numpy as np

LK, LV, N, D, H = 128, 128, 16, 256, 256
NCORES = 8
NLOC = N // NCORES  # batch items per core

# tanh(u) ~ sum_m BCOEF[m] * sin(OMEGAS[m] * u), |u| <= 6.9 (see docstring)
OMEGAS = [0.31816554069519043, 0.9786637425422668, 1.6955761909484863,
          2.454155683517456, 3.5560364723205566]
BCOEF = [1.2310866117477417, 0.30885741114616394, 0.1037391796708107,
         0.0351676307618618, 0.010901457630097866]
M = len(OMEGAS)
LMASK_NEG = -30000.0
TICKS = 2048.0  # phase resolution: 1/2048 turn per tick
IBIAS = 16384.0  # keeps fixed-point values positive; multiple of TICKS
IMASK = 2047

# ln(m) on m in [1, 2]: degree-3 least-squares fit (max err ~2e-4, well
# inside the 2e-2 output tolerance; shorter dependency chain on the tail).
_LN_COEF = None


def _ln_coef():
    global _LN_COEF
    if _LN_COEF is None:
        xs = np.linspace(1.0, 2.0, 20001)
        _LN_COEF = np.polynomial.Polynomial.fit(xs, np.log(xs), 3).convert().coef
    return _LN_COEF


_CACHE = {}


def _build_program(reps=1):
    from contextlib import ExitStack

    import concourse.bacc as bacc
    import concourse.mybir as mybir
    import concourse.tile as tile

    f32 = mybir.dt.float32
    bf16 = mybir.dt.bfloat16
    i16 = mybir.dt.int16
    i32 = mybir.dt.int32
    AF = mybir.ActivationFunctionType
    ALU = mybir.AluOpType

    cf = [float(c) for c in _ln_coef()]
    LN2 = float(np.log(2.0))

    PI = float(np.pi)
    TWOPI = float(2 * np.pi)

    nc = bacc.Bacc("TRN2", target_bir_lowering=False, debug=False)

    keyT = nc.dram_tensor("keyT", [D, NLOC, LK], bf16, kind="ExternalInput").ap()
    valT = nc.dram_tensor("valT", [D, NLOC, LV], bf16, kind="ExternalInput").ap()
    w1T = nc.dram_tensor("w1T", [D, H], bf16, kind="ExternalInput").ap()
    w2T = nc.dram_tensor("w2T", [D, H], bf16, kind="ExternalInput").ap()
    b12r = nc.dram_tensor("b12r", [1, H], f32, kind="ExternalInput").ap()
    vwr = nc.dram_tensor("vwr", [1, H], f32, kind="ExternalInput").ap()
    vbrow = nc.dram_tensor("vbrow", [1, NLOC * LV], f32, kind="ExternalInput").ap()
    lmr = nc.dram_tensor("lmr", [1, NLOC * LV], f32, kind="ExternalInput").ap()
    scores = nc.dram_tensor("scores", [LK, NLOC, LV], f32, kind="ExternalOutput").ap()

    with tile.TileContext(nc) as tc, ExitStack() as ctx:
        const = ctx.enter_context(tc.tile_pool(name="const", bufs=1 if reps == 1 else 2))
        ppre = ctx.enter_context(tc.tile_pool(name="ppre", bufs=2, space="PSUM"))
        pacc = ctx.enter_context(tc.tile_pool(name="pacc", bufs=1, space="PSUM"))
        pepi = ctx.enter_context(tc.tile_pool(name="pepi", bufs=1, space="PSUM"))
        rrpool = ctx.enter_context(tc.tile_pool(name="rrpool", bufs=3))
        trigpool = ctx.enter_context(tc.tile_pool(name="trigpool", bufs=3))
        afpool = ctx.enter_context(tc.tile_pool(name="afpool", bufs=2))
        epool = ctx.enter_context(tc.tile_pool(name="epool", bufs=2))

        for _rep in range(reps):
            # ---- input loads, spread across DMA queues ----
            keyT_sb = const.tile([128, 2, NLOC, LK], bf16)  # (d%128, dc, n, k)
            valT_sb = const.tile([128, 2, NLOC, LV], bf16)
            w1T_sb = const.tile([128, 2, H], bf16)  # (d%128, dc, h)
            w2T_sb = const.tile([128, 2, H], bf16)
            nc.sync.dma_start(out=w1T_sb, in_=w1T.rearrange("(c p) h -> p c h", p=128))
            nc.scalar.dma_start(out=w2T_sb, in_=w2T.rearrange("(c p) h -> p c h", p=128))
            nc.sync.dma_start(out=keyT_sb, in_=keyT.rearrange("(c p) n k -> p c n k", p=128))
            nc.scalar.dma_start(out=valT_sb, in_=valT.rearrange("(c p) n k -> p c n k", p=128))
            b12_sb = const.tile([1, H], f32)
            nc.gpsimd.dma_start(out=b12_sb, in_=b12r)
            vwcol = const.tile([128, 2], f32)  # v_w as per-partition cols per hc
            nc.gpsimd.dma_start(out=vwcol, in_=vwr.rearrange("o (c p) -> p (o c)", p=128))
            vb_sb = const.tile([1, NLOC * LV], f32)
            nc.gpsimd.dma_start(out=vb_sb, in_=vbrow)
            lm_sb = const.tile([1, NLOC * LV], f32)
            nc.gpsimd.dma_start(out=lm_sb, in_=lmr)

            ones = const.tile([1, 256], f32)
            nc.vector.memset(ones, 1.0)
            # Sin biases: pi/2 (direct cos for m=0) and -pi*c (maps phase
            # ticks [0, 2048) to [-pi, pi); c slightly < 1 keeps args inside
            # the table's [-pi, pi] domain).
            CSL = 1.0 - 2e-4
            pihalf = const.tile([128, 1], f32)
            nc.vector.memset(pihalf, PI / 2)
            npic = const.tile([128, 1], f32)
            nc.vector.memset(npic, -PI * CSL)

            # ln() constants for the DVE log polynomial (exp/mantissa split)
            c23 = const.tile([128, 2], i32, tag="c23")
            nc.vector.memset(c23, 23)
            cmant = const.tile([128, 2], i32, tag="cmant")
            nc.vector.memset(cmant, 0x007FFFFF)
            cexp1 = const.tile([128, 2], i32, tag="cexp1")
            nc.vector.memset(cexp1, 0x3F800000)

            # Dummy 1-element Sin so the auto-inserted sin-table load runs
            # NOW (ACT idle, under the input DMAs) instead of right before
            # the first real sin on the critical path.
            preload_scr = const.tile([128, 1], f32)
            nc.scalar.activation(preload_scr, pihalf, AF.Sin)

            # ---- prologue matmuls: kt/vt marginals in PSUM ----
            # bank layout [128 (h%128), hc*256 + n*128 + k]
            bank_kt = ppre.tile([128, 512], f32, tag="bkt")
            bank_vt = ppre.tile([128, 512], f32, tag="bvt")
            for hc in range(2):
                hsl = slice(hc * 128, (hc + 1) * 128)
                for dc in range(2):
                    nc.tensor.matmul(
                        out=bank_kt[:, hc * 256 : (hc + 1) * 256],
                        lhsT=w1T_sb[:, dc, hsl],
                        rhs=keyT_sb[:, dc],
                        start=(dc == 0),
                        stop=(dc == 1),
                    )
            # kt half of the combined bf16 tile, copied as soon as kt lands
            kv_sb = const.tile([128, 1024], bf16)
            nc.vector.tensor_copy(kv_sb[:, 0:512], bank_kt)

            # ---- b12 = (w1_b + w2_b) as per-partition cols [128, 2] ----
            b12_ps = pepi.tile([128, 2], f32, tag="b12ps")
            for hc in range(2):
                nc.tensor.matmul(
                    out=b12_ps[:, hc : hc + 1],
                    lhsT=b12_sb[:, hc * 128 : (hc + 1) * 128],
                    rhs=ones[:, :1],
                    start=True,
                    stop=True,
                )
            b12c_sb = const.tile([128, 2], f32)
            nc.vector.tensor_copy(b12c_sb, b12_ps)

            for hc in range(2):
                hsl = slice(hc * 128, (hc + 1) * 128)
                for dc in range(2):
                    nc.tensor.matmul(
                        out=bank_vt[:, hc * 256 : (hc + 1) * 256],
                        lhsT=w2T_sb[:, dc, hsl],
                        rhs=valT_sb[:, dc],
                        start=(dc == 0),
                        stop=(dc == 1),
                    )
            for hc in range(2):
                nc.vector.tensor_scalar_add(
                    kv_sb[:, 512 + hc * 256 : 512 + (hc + 1) * 256],
                    bank_vt[:, hc * 256 : (hc + 1) * 256],
                    b12c_sb[:, hc : hc + 1],
                )

            # ---- xi accumulator seeded with v_b; log-mask replica ----
            xi_t = pacc.tile([LK, NLOC * LV], f32, tag="xi")
            nc.tensor.matmul(
                out=xi_t, lhsT=ones[:, :LK], rhs=vb_sb, start=True, stop=True
            )
            lmrep_ps = pepi.tile([LK, NLOC * LV], f32, tag="lmrep")
            nc.tensor.matmul(
                out=lmrep_ps, lhsT=ones[:, :LK], rhs=lm_sb, start=True, stop=True
            )
            lmrep = const.tile([LK, NLOC * LV], f32)

            def mods(m):
                # Range reduction in int16 fixed point (HW has no mod op):
                # v = floor(x*s_ticks + 16384.5) then v & 2047 == phase in
                # 1/2048-turn ticks (the +16384 bias keeps v positive so
                # truncation == floor, and vanishes under the mask). Cos
                # branch pre-adds a quarter turn (512 ticks). ACT Sin maps
                # ticks to [-pi, pi) via scale/bias; the resulting -sin/-cos
                # signs cancel pairwise in the rank-2 products.
                s = OMEGAS[m] / TWOPI * TICKS
                iv = rrpool.tile([128, 2048], i16, tag="iv")
                nc.vector.tensor_scalar(
                    out=iv[:, 0:1024], in0=kv_sb, scalar1=s,
                    scalar2=IBIAS + 0.5, op0=ALU.mult, op1=ALU.add,
                )
                nc.vector.tensor_scalar(
                    out=iv[:, 1024:2048], in0=kv_sb, scalar1=s,
                    scalar2=IBIAS + 0.5 + TICKS / 4, op0=ALU.mult, op1=ALU.add,
                )
                rr = rrpool.tile([128, 2048], i16, tag="rr")
                nc.vector.tensor_scalar(
                    out=rr, in0=iv, scalar1=IMASK, scalar2=None, op0=ALU.bitwise_and
                )
                return rr

            def epilogue_exp(n):
                # masked row sums S on the fly; both n's exps are emitted
                # back-to-back so only ONE exp-table load happens (ln is a
                # DVE polynomial: no ln-table, no table thrashing).
                ksl = slice(n * LV, (n + 1) * LV)
                xe = epool.tile([LK, LV], f32, tag=f"xe{n}")
                nc.vector.tensor_tensor(xe, xi_t[:, ksl], lmrep[:, ksl], op=ALU.add)
                escr = epool.tile([LK, LV], f32, tag=f"escr{n}")
                S = epool.tile([LK, 1], f32, tag=f"S{n}")
                nc.scalar.activation(escr, xe, AF.Exp, accum_out=S)
                return S

            def epilogue_log(n, S):
                # per-n chain, pure DVE (doesn't block the other n's exp):
                # ln(Sg) via exponent/mantissa split + deg-3 Estrin.
                ksl = slice(n * LV, (n + 1) * LV)
                Sg = epool.tile([LK, 1], f32, tag=f"Sg{n}")
                # Sg = (S == 0 ? 1 : 0) + S  == where(S==0, 1, S)
                nc.vector.scalar_tensor_tensor(
                    out=Sg, in0=S, scalar=0.0, in1=S, op0=ALU.is_equal, op1=ALU.add
                )
                xu = Sg.bitcast(i32)
                e_i = epool.tile([LK, 1], i32, tag=f"e_i{n}")
                nc.vector.tensor_tensor(e_i, xu, c23[:, 0:1], op=ALU.logical_shift_right)
                e_f = epool.tile([LK, 1], f32, tag=f"e_f{n}")
                nc.vector.tensor_copy(e_f, e_i)  # int -> float convert
                esc = epool.tile([LK, 1], f32, tag=f"esc{n}")
                nc.vector.tensor_scalar(
                    out=esc, in0=e_f, scalar1=LN2, scalar2=-127.0 * LN2,
                    op0=ALU.mult, op1=ALU.add,
                )
                m_i = epool.tile([LK, 1], i32, tag=f"m_i{n}")
                nc.vector.tensor_tensor(m_i, xu, cmant[:, 0:1], op=ALU.bitwise_and)
                nc.vector.tensor_tensor(m_i, m_i, cexp1[:, 0:1], op=ALU.bitwise_or)
                mm = m_i.bitcast(f32)  # mantissa in [1, 2)
                m2 = epool.tile([LK, 1], f32, tag=f"m2{n}")
                nc.vector.tensor_tensor(m2, mm, mm, op=ALU.mult)
                u = epool.tile([LK, 1], f32, tag=f"u{n}")
                nc.vector.tensor_scalar(
                    out=u, in0=mm, scalar1=cf[1], scalar2=cf[0], op0=ALU.mult, op1=ALU.add
                )
                vq = epool.tile([LK, 1], f32, tag=f"vq{n}")
                nc.vector.tensor_scalar(
                    out=vq, in0=mm, scalar1=cf[3], scalar2=cf[2], op0=ALU.mult, op1=ALU.add
                )
                t = epool.tile([LK, 1], f32, tag=f"t{n}")
                nc.vector.scalar_tensor_tensor(
                    out=t, in0=m2, scalar=1.0, in1=vq, op0=ALU.mult, op1=ALU.mult
                )
                nc.vector.tensor_tensor(t, t, u, op=ALU.add)
                logS = epool.tile([LK, 1], f32, tag=f"logS{n}")
                nc.vector.tensor_tensor(logS, t, esc, op=ALU.add)
                sc = epool.tile([LK, LV], f32, tag=f"sc{n}")
                nc.vector.tensor_scalar_sub(sc, xi_t[:, ksl], logS)
                # one output DMA per HW queue: parallel trigger paths
                (nc.sync if n == 0 else nc.scalar).dma_start(
                    out=scores[:, n, :], in_=sc
                )

            # ---- main loop over sine terms (m=0 direct, m>=1 reduced;
            # reductions prefetched 2 deep) ----
            Ss = {}
            rrs = {1: mods(1), 2: mods(2)}
            # deferred: lmrep not needed until the epilogue, so don't let its
            # copy occupy DVE while the first reductions are on the clock
            nc.vector.tensor_copy(lmrep, lmrep_ps)
            for m in range(M):
                trig = trigpool.tile([128, 2048], bf16, tag="trig")
                # trig[:, f*1024 + side*512 + hc*256 + n*128 + k]:
                #   f=0: sin(w x), f=1: cos(w x)
                if m == 0:
                    # |w0 x| <= 1.18, |w0 x + pi/2| <= 2.76 < pi: no reduction.
                    # Read kt/vt straight from PSUM (f32) so ACT starts while
                    # DVE is still building kv_sb; vt's b12 bias is folded
                    # into the per-partition sin bias (per h-chunk).
                    nc.scalar.activation(
                        trig[:, 0:512], bank_kt, AF.Sin, scale=OMEGAS[0]
                    )
                    nc.scalar.activation(
                        trig[:, 1024:1536], bank_kt, AF.Sin,
                        scale=OMEGAS[0], bias=pihalf,
                    )
                    # vt side from kv_sb (b12 already folded in): one wide
                    # instr per phase instead of two per-hc PSUM reads
                    nc.scalar.activation(
                        trig[:, 512:1024], kv_sb[:, 512:1024], AF.Sin,
                        scale=OMEGAS[0],
                    )
                    nc.scalar.activation(
                        trig[:, 1536:2048], kv_sb[:, 512:1024], AF.Sin,
                        scale=OMEGAS[0], bias=pihalf,
                    )
                else:
                    # halves hold sin/cos phase ticks; outputs are
                    # -sin(w x), -cos(w x) (signs cancel in the products)
                    nc.scalar.activation(
                        trig, rrs.pop(m), AF.Sin,
                        scale=TWOPI / TICKS * CSL, bias=npic,
                    )
                if m >= 1 and m + 2 < M:
                    rrs[m + 2] = mods(m + 2)

                af = afpool.tile([128, 2, 2, NLOC, 128], bf16, tag="af")
                # af[:, f, hc] = vwb * trig[f, v-side, hc] — one op per hc
                # spanning both f (strided AP); pairs with trig[1-f, k-side].
                trig_v = trig.rearrange(
                    "p (f s c n k) -> p f s c n k", f=2, s=2, c=2, n=NLOC
                )
                for hc in range(2):
                    nc.vector.tensor_scalar(
                        out=af[:, :, hc],
                        in0=trig_v[:, :, 1, hc],
                        scalar1=vwcol[:, hc : hc + 1],
                        scalar2=BCOEF[m],
                        op0=ALU.mult,
                        op1=ALU.mult,
                    )
                for n in range(NLOC):
                    for hc in range(2):
                        for f in range(2):
                            lo = f * 1024 + hc * 256 + n * 128  # k-side (side=0)
                            nc.tensor.matmul(
                                out=xi_t[:, n * LV : (n + 1) * LV],
                                lhsT=trig[:, lo : lo + 128],
                                rhs=af[:, 1 - f, hc, n],
                                start=False,
                                stop=(m == M - 1) and hc == 1 and f == 1,
                                skip_group_check=True,
                            )
                    if m == M - 1:
                        Ss[n] = epilogue_exp(n)
                if m == M - 1:
                    for n in range(NLOC):
                        epilogue_log(n, Ss[n])

    nc.compile()
    return nc


def _get_program(reps=1):
    if reps not in _CACHE:
        _CACHE[reps] = _build_program(reps)
    return _CACHE[reps]


def _make_in_maps(key, value, mask, w1_w, w1_b, w2_w, w2_b, v_w, v_b):
    import ml_dtypes

    bf = ml_dtypes.bfloat16
    key = np.asarray(key, dtype=np.float32)
    value = np.asarray(value, dtype=np.float32)
    mask_i = np.asarray(mask)
    w1T_np = np.ascontiguousarray(np.asarray(w1_w, np.float32).T).astype(bf)  # [D, H]
    w2T_np = np.ascontiguousarray(np.asarray(w2_w, np.float32).T).astype(bf)
    b12r_np = (np.asarray(w1_b, np.float32) + np.asarray(w2_b, np.float32)).reshape(1, H)
    vwr_np = np.asarray(v_w, np.float32).reshape(1, H)
    vb_np = np.full((1, NLOC * LV), np.float32(np.asarray(v_b).reshape(-1)[0]), np.float32)

    in_maps = []
    for c in range(NCORES):
        ns = slice(c * NLOC, (c + 1) * NLOC)
        keyT_c = np.ascontiguousarray(key[:, ns, :].transpose(2, 1, 0)).astype(bf)
        valT_c = np.ascontiguousarray(value[:, ns, :].transpose(2, 1, 0)).astype(bf)
        # log-mask row, layout n*LV + v: 0 where mask==1 else big negative
        lm_c = np.where(
            mask_i[:, ns].T.astype(bool), np.float32(0.0), np.float32(LMASK_NEG)
        ).reshape(1, NLOC * LV).astype(np.float32)
        in_maps.append(
            {
                "keyT": keyT_c,
                "valT": valT_c,
                "w1T": w1T_np,
                "w2T": w2T_np,
                "b12r": b12r_np,
                "vwr": vwr_np,
                "vbrow": vb_np,
                "lmr": np.ascontiguousarray(lm_c),
            }
        )
    return in_maps


def kernel(**inputs):
    from concourse.bass_utils import run_bass_kernel_spmd

    nc = _get_program()
    in_maps = _make_in_maps(**inputs)
    res = run_bass_kernel_spmd(nc, in_maps, core_ids=list(range(NCORES)))
    out = np.empty((LK, N, LV), np.float32)
    for c in range(NCORES):
        out[:, c * NLOC : (c + 1) * NLOC, :] = res.results[c]["scores"].reshape(LK, NLOC, LV)
    return out


# revision 65
# speedup vs baseline: 3.8820x; 3.8820x over previous
"""PointerNet additive-attention scores kernel for Trainium2 (8 NeuronCores).

Math (reference):
    kt[k,n,h] = key[k,n,:] @ w1_w[h,:]
    vt[v,n,h] = value[v,n,:] @ w2_w[h,:] + (w1_b[h] + w2_b[h])
    xi[k,v,n] = sum_h v_w[h] * tanh(kt[k,n,h] + vt[v,n,h]) + v_b
    S[k,n]    = sum_v exp(xi[k,v,n]) * mask[v,n];  S==0 -> 1
    out[k,n,v] = xi[k,v,n] - log(S[k,n])

Key trick: tanh(u) on the data range |u| <= 6.9 is approximated by a sum of
M=5 sines, tanh(u) ~ sum_m b_m sin(w_m u), fitted (weighted by the empirical
u-density) to ~9e-4. Since sin(w(k+v)) = sin(wk)cos(wv) + cos(wk)sin(wv),
the (Lk x Lv x H) inner broadcast+tanh collapses to 2M rank-H matmuls on the
PE array. Per-element work drops from Lk*Lv*N*H tanh evals (the ACT-engine
roofline of the direct kernel, ~25 us) to (Lk+Lv)*N*H*2M trig evals.

Per-core dataflow (data-parallel over batch N, NLOC=2 items/core):
  - PE: kt/vt marginals ([128h, (n k)] per h-chunk) from bf16 inputs.
  - DVE range reduction in int16 fixed point (HW has no mod ALU op):
    v = floor(x*w/2pi*2048 + 16384.5); v & 2047 is the phase in
    1/2048-turn ticks (positive bias makes truncation floor and vanishes
    under the mask); the cos branch pre-adds 512 ticks. Three 2-byte DVE
    ops per m. m=0 needs no reduction (|w0 x| + pi/2 < pi): its sin/cos
    read the kt PSUM bank / vt kv-half directly, starting ACT while the
    SBUF copies still run (a dummy 1-element Sin hoists the sin-table
    load into the DMA shadow).
  - ACT (Sin table): one [128, 2048] instruction per m>=1 maps ticks to
    [-pi, pi) via scale/bias and yields -sin/-cos of w_m x for both
    sides, h-chunks, and batch items; the signs cancel pairwise in the
    rank-2 products.
  - DVE: fold v_w[h] * b_m into the v-side factors (one tensor_scalar per
    h-chunk spanning both phases via a strided AP).
  - PE: xi[:, n, :] += s_k^T (vwb c_v) + c_k^T (vwb s_v), accumulated
    in one PSUM bank over all (m, hc), seeded with v_b via a c=1 matmul.
  - Epilogue: masked sum via log-mask add (0 / -30000) + Exp with
    accum_out (row sums on the fly, one exp-table load overlapped with
    the final matmuls), S==0 guard, then per-batch-item pure-DVE ln
    (exponent/mantissa split + deg-3 Estrin; no ln-table load), per-
    partition subtract, one output DMA per HW queue.
"""

import numpy as np

LK, LV, N, D, H = 128, 128, 16, 256, 256
NCORES = 8
NLOC = N // NCORES  # batch items per core

# tanh(u) ~ sum_m BCOEF[m] * sin(OMEGAS[m] * u), |u| <= 6.9 (see docstring).
# 4 sines suffice: coefficients re-solved with the 5th (b~0.011) dropped;
# end-to-end max rel err 3.8e-3 vs the 2e-2 gate.
OMEGAS = [0.31816554069519043, 0.9786637425422668, 1.6955761909484863,
          2.454155683517456]
BCOEF = [1.231811761856079, 0.3134132921695709, 0.09274235367774963,
         0.04794609546661377]
M = len(OMEGAS)
LMASK_NEG = -30000.0
TICKS = 2048.0  # phase resolution: 1/2048 turn per tick
IBIAS = 16384.0  # keeps fixed-point values positive; multiple of TICKS
IMASK = 2047

# ln(m) on m in [1, 2]: degree-3 least-squares fit (max err ~2e-4, well
# inside the 2e-2 output tolerance; shorter dependency chain on the tail).
_LN_COEF = None


def _ln_coef():
    global _LN_COEF
    if _LN_COEF is None:
        xs = np.linspace(1.0, 2.0, 20001)
        _LN_COEF = np.polynomial.Polynomial.fit(xs, np.log(xs), 3).convert().coef
    return _LN_COEF


_CACHE = {}


def _build_program(reps=1):
    from contextlib import ExitStack

    import concourse.bacc as bacc
    import concourse.mybir as mybir
    import concourse.tile as tile

    f32 = mybir.dt.float32
    bf16 = mybir.dt.bfloat16
    i16 = mybir.dt.int16
    i32 = mybir.dt.int32
    AF = mybir.ActivationFunctionType
    ALU = mybir.AluOpType

    cf = [float(c) for c in _ln_coef()]
    LN2 = float(np.log(2.0))

    PI = float(np.pi)
    TWOPI = float(2 * np.pi)

    nc = bacc.Bacc("TRN2", target_bir_lowering=False, debug=False)

    keyT = nc.dram_tensor("keyT", [D, NLOC, LK], bf16, kind="ExternalInput").ap()
    valT = nc.dram_tensor("valT", [D, NLOC, LV], bf16, kind="ExternalInput").ap()
    w1T = nc.dram_tensor("w1T", [D, H], bf16, kind="ExternalInput").ap()
    w2T = nc.dram_tensor("w2T", [D, H], bf16, kind="ExternalInput").ap()
    b12r = nc.dram_tensor("b12r", [1, H], f32, kind="ExternalInput").ap()
    vwr = nc.dram_tensor("vwr", [1, H], f32, kind="ExternalInput").ap()
    vbrow = nc.dram_tensor("vbrow", [1, NLOC * LV], f32, kind="ExternalInput").ap()
    lmr = nc.dram_tensor("lmr", [1, NLOC * LV], f32, kind="ExternalInput").ap()
    scores = nc.dram_tensor("scores", [LK, NLOC, LV], f32, kind="ExternalOutput").ap()

    with tile.TileContext(nc) as tc, ExitStack() as ctx:
        const = ctx.enter_context(tc.tile_pool(name="const", bufs=1 if reps == 1 else 2))
        ppre = ctx.enter_context(tc.tile_pool(name="ppre", bufs=2, space="PSUM"))
        pacc = ctx.enter_context(tc.tile_pool(name="pacc", bufs=1, space="PSUM"))
        pepi = ctx.enter_context(tc.tile_pool(name="pepi", bufs=1, space="PSUM"))
        rrpool = ctx.enter_context(tc.tile_pool(name="rrpool", bufs=3))
        trigpool = ctx.enter_context(tc.tile_pool(name="trigpool", bufs=3))
        afpool = ctx.enter_context(tc.tile_pool(name="afpool", bufs=2))
        epool = ctx.enter_context(tc.tile_pool(name="epool", bufs=2))

        for _rep in range(reps):
            # ---- input loads, spread across DMA queues ----
            keyT_sb = const.tile([128, 2, NLOC, LK], bf16)  # (d%128, dc, n, k)
            valT_sb = const.tile([128, 2, NLOC, LV], bf16)
            w1T_sb = const.tile([128, 2, H], bf16)  # (d%128, dc, h)
            w2T_sb = const.tile([128, 2, H], bf16)
            nc.sync.dma_start(out=w1T_sb, in_=w1T.rearrange("(c p) h -> p c h", p=128))
            nc.scalar.dma_start(out=w2T_sb, in_=w2T.rearrange("(c p) h -> p c h", p=128))
            nc.sync.dma_start(out=keyT_sb, in_=keyT.rearrange("(c p) n k -> p c n k", p=128))
            nc.scalar.dma_start(out=valT_sb, in_=valT.rearrange("(c p) n k -> p c n k", p=128))
            b12_sb = const.tile([1, H], f32)
            nc.gpsimd.dma_start(out=b12_sb, in_=b12r)
            vwcol = const.tile([128, 2], f32)  # v_w as per-partition cols per hc
            nc.gpsimd.dma_start(out=vwcol, in_=vwr.rearrange("o (c p) -> p (o c)", p=128))
            vb_sb = const.tile([1, NLOC * LV], f32)
            nc.gpsimd.dma_start(out=vb_sb, in_=vbrow)
            lm_sb = const.tile([1, NLOC * LV], f32)
            nc.gpsimd.dma_start(out=lm_sb, in_=lmr)

            ones = const.tile([1, 256], f32)
            nc.vector.memset(ones, 1.0)
            # Sin biases: pi/2 (direct cos for m=0) and -pi*c (maps phase
            # ticks [0, 2048) to [-pi, pi); c slightly < 1 keeps args inside
            # the table's [-pi, pi] domain).
            CSL = 1.0 - 2e-4
            pihalf = const.tile([128, 1], f32)
            nc.vector.memset(pihalf, PI / 2)
            npic = const.tile([128, 1], f32)
            nc.vector.memset(npic, -PI * CSL)

            # ln() constants for the DVE log polynomial (exp/mantissa split)
            c23 = const.tile([128, 2], i32, tag="c23")
            nc.vector.memset(c23, 23)
            cmant = const.tile([128, 2], i32, tag="cmant")
            nc.vector.memset(cmant, 0x007FFFFF)
            cexp1 = const.tile([128, 2], i32, tag="cexp1")
            nc.vector.memset(cexp1, 0x3F800000)

            # Dummy 1-element Sin so the auto-inserted sin-table load runs
            # NOW (ACT idle, under the input DMAs) instead of right before
            # the first real sin on the critical path.
            preload_scr = const.tile([128, 1], f32)
            nc.scalar.activation(preload_scr, pihalf, AF.Sin)

            # ---- prologue matmuls: kt/vt marginals in PSUM ----
            # bank layout [128 (h%128), hc*256 + n*128 + k]
            bank_kt = ppre.tile([128, 512], f32, tag="bkt")
            bank_vt = ppre.tile([128, 512], f32, tag="bvt")
            for hc in range(2):
                hsl = slice(hc * 128, (hc + 1) * 128)
                for dc in range(2):
                    nc.tensor.matmul(
                        out=bank_kt[:, hc * 256 : (hc + 1) * 256],
                        lhsT=w1T_sb[:, dc, hsl],
                        rhs=keyT_sb[:, dc],
                        start=(dc == 0),
                        stop=(dc == 1),
                    )
            # kt half of the combined bf16 tile, copied as soon as kt lands
            kv_sb = const.tile([128, 1024], bf16)
            nc.vector.tensor_copy(kv_sb[:, 0:512], bank_kt)

            # ---- b12 = (w1_b + w2_b) as per-partition cols [128, 2] ----
            b12_ps = pepi.tile([128, 2], f32, tag="b12ps")
            for hc in range(2):
                nc.tensor.matmul(
                    out=b12_ps[:, hc : hc + 1],
                    lhsT=b12_sb[:, hc * 128 : (hc + 1) * 128],
                    rhs=ones[:, :1],
                    start=True,
                    stop=True,
                )
            b12c_sb = const.tile([128, 2], f32)
            nc.vector.tensor_copy(b12c_sb, b12_ps)

            for hc in range(2):
                hsl = slice(hc * 128, (hc + 1) * 128)
                for dc in range(2):
                    nc.tensor.matmul(
                        out=bank_vt[:, hc * 256 : (hc + 1) * 256],
                        lhsT=w2T_sb[:, dc, hsl],
                        rhs=valT_sb[:, dc],
                        start=(dc == 0),
                        stop=(dc == 1),
                    )
            for hc in range(2):
                nc.vector.tensor_scalar_add(
                    kv_sb[:, 512 + hc * 256 : 512 + (hc + 1) * 256],
                    bank_vt[:, hc * 256 : (hc + 1) * 256],
                    b12c_sb[:, hc : hc + 1],
                )

            # ---- xi accumulator seeded with v_b; log-mask replica ----
            xi_t = pacc.tile([LK, NLOC * LV], f32, tag="xi")
            nc.tensor.matmul(
                out=xi_t, lhsT=ones[:, :LK], rhs=vb_sb, start=True, stop=True
            )
            lmrep_ps = pepi.tile([LK, NLOC * LV], f32, tag="lmrep")
            nc.tensor.matmul(
                out=lmrep_ps, lhsT=ones[:, :LK], rhs=lm_sb, start=True, stop=True
            )
            lmrep = const.tile([LK, NLOC * LV], f32)

            def mods(m):
                # Range reduction in int16 fixed point (HW has no mod op):
                # v = floor(x*s_ticks + 16384.5) then v & 2047 == phase in
                # 1/2048-turn ticks (the +16384 bias keeps v positive so
                # truncation == floor, and vanishes under the mask). Cos
                # branch pre-adds a quarter turn (512 ticks). ACT Sin maps
                # ticks to [-pi, pi) via scale/bias; the resulting -sin/-cos
                # signs cancel pairwise in the rank-2 products.
                s = OMEGAS[m] / TWOPI * TICKS
                iv = rrpool.tile([128, 2048], i16, tag="iv")
                nc.vector.tensor_scalar(
                    out=iv[:, 0:1024], in0=kv_sb, scalar1=s,
                    scalar2=IBIAS + 0.5, op0=ALU.mult, op1=ALU.add,
                )
                nc.vector.tensor_scalar(
                    out=iv[:, 1024:2048], in0=kv_sb, scalar1=s,
                    scalar2=IBIAS + 0.5 + TICKS / 4, op0=ALU.mult, op1=ALU.add,
                )
                rr = rrpool.tile([128, 2048], i16, tag="rr")
                nc.vector.tensor_scalar(
                    out=rr, in0=iv, scalar1=IMASK, scalar2=None, op0=ALU.bitwise_and
                )
                return rr

            def epilogue_exp(n):
                # masked row sums S on the fly; both n's exps are emitted
                # back-to-back so only ONE exp-table load happens (ln is a
                # DVE polynomial: no ln-table, no table thrashing).
                ksl = slice(n * LV, (n + 1) * LV)
                xe = epool.tile([LK, LV], f32, tag=f"xe{n}")
                nc.vector.tensor_tensor(xe, xi_t[:, ksl], lmrep[:, ksl], op=ALU.add)
                escr = epool.tile([LK, LV], f32, tag=f"escr{n}")
                S = epool.tile([LK, 1], f32, tag=f"S{n}")
                nc.scalar.activation(escr, xe, AF.Exp, accum_out=S)
                return S

            def epilogue_log(n, S):
                # per-n chain, pure DVE (doesn't block the other n's exp):
                # ln(Sg) via exponent/mantissa split + deg-3 Estrin.
                ksl = slice(n * LV, (n + 1) * LV)
                Sg = epool.tile([LK, 1], f32, tag=f"Sg{n}")
                # Sg = (S == 0 ? 1 : 0) + S  == where(S==0, 1, S)
                nc.vector.scalar_tensor_tensor(
                    out=Sg, in0=S, scalar=0.0, in1=S, op0=ALU.is_equal, op1=ALU.add
                )
                xu = Sg.bitcast(i32)
                e_i = epool.tile([LK, 1], i32, tag=f"e_i{n}")
                nc.vector.tensor_tensor(e_i, xu, c23[:, 0:1], op=ALU.logical_shift_right)
                e_f = epool.tile([LK, 1], f32, tag=f"e_f{n}")
                nc.vector.tensor_copy(e_f, e_i)  # int -> float convert
                esc = epool.tile([LK, 1], f32, tag=f"esc{n}")
                nc.vector.tensor_scalar(
                    out=esc, in0=e_f, scalar1=LN2, scalar2=-127.0 * LN2,
                    op0=ALU.mult, op1=ALU.add,
                )
                m_i = epool.tile([LK, 1], i32, tag=f"m_i{n}")
                nc.vector.tensor_tensor(m_i, xu, cmant[:, 0:1], op=ALU.bitwise_and)
                nc.vector.tensor_tensor(m_i, m_i, cexp1[:, 0:1], op=ALU.bitwise_or)
                mm = m_i.bitcast(f32)  # mantissa in [1, 2)
                m2 = epool.tile([LK, 1], f32, tag=f"m2{n}")
                nc.vector.tensor_tensor(m2, mm, mm, op=ALU.mult)
                u = epool.tile([LK, 1], f32, tag=f"u{n}")
                nc.vector.tensor_scalar(
                    out=u, in0=mm, scalar1=cf[1], scalar2=cf[0], op0=ALU.mult, op1=ALU.add
                )
                vq = epool.tile([LK, 1], f32, tag=f"vq{n}")
                nc.vector.tensor_scalar(
                    out=vq, in0=mm, scalar1=cf[3], scalar2=cf[2], op0=ALU.mult, op1=ALU.add
                )
                t = epool.tile([LK, 1], f32, tag=f"t{n}")
                nc.vector.scalar_tensor_tensor(
                    out=t, in0=m2, scalar=1.0, in1=vq, op0=ALU.mult, op1=ALU.mult
                )
                nc.vector.tensor_tensor(t, t, u, op=ALU.add)
                logS = epool.tile([LK, 1], f32, tag=f"logS{n}")
                nc.vector.tensor_tensor(logS, t, esc, op=ALU.add)
                sc = epool.tile([LK, LV], f32, tag=f"sc{n}")
                nc.vector.tensor_scalar_sub(sc, xi_t[:, ksl], logS)
                # one output DMA per HW queue: parallel trigger paths
                (nc.sync if n == 0 else nc.scalar).dma_start(
                    out=scores[:, n, :], in_=sc
                )

            # ---- main loop over sine terms (m=0 direct, m>=1 reduced;
            # reductions prefetched 2 deep) ----
            Ss = {}
            rrs = {1: mods(1), 2: mods(2)}
            # deferred: lmrep not needed until the epilogue, so don't let its
            # copy occupy DVE while the first reductions are on the clock
            nc.vector.tensor_copy(lmrep, lmrep_ps)
            for m in range(M):
                trig = trigpool.tile([128, 2048], bf16, tag="trig")
                # trig[:, f*1024 + side*512 + hc*256 + n*128 + k]:
                #   f=0: sin(w x), f=1: cos(w x)
                if m == 0:
                    # |w0 x| <= 1.18, |w0 x + pi/2| <= 2.76 < pi: no reduction.
                    # Read kt/vt straight from PSUM (f32) so ACT starts while
                    # DVE is still building kv_sb; vt's b12 bias is folded
                    # into the per-partition sin bias (per h-chunk).
                    nc.scalar.activation(
                        trig[:, 0:512], bank_kt, AF.Sin, scale=OMEGAS[0]
                    )
                    nc.scalar.activation(
                        trig[:, 1024:1536], bank_kt, AF.Sin,
                        scale=OMEGAS[0], bias=pihalf,
                    )
                    # vt side from kv_sb (b12 already folded in): one wide
                    # instr per phase instead of two per-hc PSUM reads
                    nc.scalar.activation(
                        trig[:, 512:1024], kv_sb[:, 512:1024], AF.Sin,
                        scale=OMEGAS[0],
                    )
                    nc.scalar.activation(
                        trig[:, 1536:2048], kv_sb[:, 512:1024], AF.Sin,
                        scale=OMEGAS[0], bias=pihalf,
                    )
                else:
                    # halves hold sin/cos phase ticks; outputs are
                    # -sin(w x), -cos(w x) (signs cancel in the products)
                    nc.scalar.activation(
                        trig, rrs.pop(m), AF.Sin,
                        scale=TWOPI / TICKS * CSL, bias=npic,
                    )
                if m >= 1 and m + 2 < M:
                    rrs[m + 2] = mods(m + 2)

                af = afpool.tile([128, 2, 2, NLOC, 128], bf16, tag="af")
                # af[:, f, hc] = vwb * trig[f, v-side, hc] — one op per hc
                # spanning both f (strided AP); pairs with trig[1-f, k-side].
                trig_v = trig.rearrange(
                    "p (f s c n k) -> p f s c n k", f=2, s=2, c=2, n=NLOC
                )
                for hc in range(2):
                    nc.vector.tensor_scalar(
                        out=af[:, :, hc],
                        in0=trig_v[:, :, 1, hc],
                        scalar1=vwcol[:, hc : hc + 1],
                        scalar2=BCOEF[m],
                        op0=ALU.mult,
                        op1=ALU.mult,
                    )
                for n in range(NLOC):
                    for hc in range(2):
                        for f in range(2):
                            lo = f * 1024 + hc * 256 + n * 128  # k-side (side=0)
                            nc.tensor.matmul(
                                out=xi_t[:, n * LV : (n + 1) * LV],
                                lhsT=trig[:, lo : lo + 128],
                                rhs=af[:, 1 - f, hc, n],
                                start=False,
                                stop=(m == M - 1) and hc == 1 and f == 1,
                                skip_group_check=True,
                            )
                    if m == M - 1:
                        Ss[n] = epilogue_exp(n)
                if m == M - 1:
                    for n in range(NLOC):
                        epilogue_log(n, Ss[n])

    nc.compile()
    return nc


def _get_program(reps=1):
    if reps not in _CACHE:
        _CACHE[reps] = _build_program(reps)
    return _CACHE[reps]


def _make_in_maps(key, value, mask, w1_w, w1_b, w2_w, w2_b, v_w, v_b):
    import ml_dtypes

    bf = ml_dtypes.bfloat16
    key = np.asarray(key, dtype=np.float32)
    value = np.asarray(value, dtype=np.float32)
    mask_i = np.asarray(mask)
    w1T_np = np.ascontiguousarray(np.asarray(w1_w, np.float32).T).astype(bf)  # [D, H]
    w2T_np = np.ascontiguousarray(np.asarray(w2_w, np.float32).T).astype(bf)
    b12r_np = (np.asarray(w1_b, np.float32) + np.asarray(w2_b, np.float32)).reshape(1, H)
    vwr_np = np.asarray(v_w, np.float32).reshape(1, H)
    vb_np = np.full((1, NLOC * LV), np.float32(np.asarray(v_b).reshape(-1)[0]), np.float32)

    in_maps = []
    for c in range(NCORES):
        ns = slice(c * NLOC, (c + 1) * NLOC)
        keyT_c = np.ascontiguousarray(key[:, ns, :].transpose(2, 1, 0)).astype(bf)
        valT_c = np.ascontiguousarray(value[:, ns, :].transpose(2, 1, 0)).astype(bf)
        # log-mask row, layout n*LV + v: 0 where mask==1 else big negative
        lm_c = np.where(
            mask_i[:, ns].T.astype(bool), np.float32(0.0), np.float32(LMASK_NEG)
        ).reshape(1, NLOC * LV).astype(np.float32)
        in_maps.append(
            {
                "keyT": keyT_c,
                "valT": valT_c,
                "w1T": w1T_np,
                "w2T": w2T_np,
                "b12r": b12r_np,
                "vwr": vwr_np,
                "vbrow": vb_np,
                "lmr": np.ascontiguousarray(lm_c),
            }
        )
    return in_maps


def kernel(**inputs):
    from concourse.bass_utils import run_bass_kernel_spmd

    nc = _get_program()
    in_maps = _make_in_maps(**inputs)
    res = run_bass_kernel_spmd(nc, in_maps, core_ids=list(range(NCORES)))
    out = np.empty((LK, N, LV), np.float32)
    for c in range(NCORES):
        out[:, c * NLOC : (c + 1) * NLOC, :] = res.results[c]["scores"].reshape(LK, NLOC, LV)
    return out
